# revision 8
# baseline (speedup 1.0000x reference)
"""Multi-head attention (B=2, S=2048, D=1024, H=16) on 8 Trainium2 NeuronCores.

Sharding: core = (batch, head-group-of-4). Cores 0-3 -> b=0, head groups 0-3;
cores 4-7 -> b=1. Column-parallel Wq/Wk/Wv (each core projects its 256 head
dims), row-parallel Wo (each core produces a partial [S, D] output summed on
host). The attention-probability output attn [B,H,S,S] is written directly by
each core for its 4 (b,h) pairs.

Fast path: if the mask equals the causal tril mask, only lower-triangle blocks
are computed; upper-triangle stays zero via the runtime's pre-zeroed output
buffers. Masking is applied by accumulating a -8e9 bias into the scores PSUM
through an identity matmul, so exp() produces exact zeros and row sums (taken
via the activation accum_out) are correct. A generic path streams a full
[S, S] additive bias built from the mask on the host.

Matmuls run in float32r (~11-bit mantissa, 1 cyc/row at N>=256); the
attn-probability tiles are float32r end-to-end (PE transpose at 1.5 cyc/row);
the attn^T @ v context path runs in bf16. Row normalization runs on the
otherwise-idle GPSIMD engine; PSUM->SBUF traffic runs on DVE.
"""
import sys

for _p in ("/opt/trn_rl_repo", "/root/.axon_site/_ro/trn_rl_repo"):
    if _p not in sys.path:
        sys.path.append(_p)

import numpy as np
import concourse.bacc as bacc
import concourse.tile as tile
from concourse import mybir
from concourse.bass_utils import run_bass_kernel_spmd

B, S, D, H = 2, 2048, 1024, 16
DK = D // H  # 64
N_CORES = 8
HC = H // (N_CORES // B)  # 4 heads per core
HD = HC * DK  # 256 head dims per core
SB = S // 128  # 16 q/k blocks
F32 = mybir.dt.float32
F32R = mybir.dt.float32r
BF16 = mybir.dt.bfloat16
AF = mybir.ActivationFunctionType
AX = mybir.AxisListType

TRACE = False
NORM_ENGINE = "gpsimd"  # "gpsimd" | "vector" | "scalar"
LAST_RES = None
_PROG_CACHE = {}


def _build_program(causal: bool):
    nc = bacc.Bacc("TRN2", target_bir_lowering=False, debug=False,
                   num_devices=N_CORES)

    qt_d = nc.dram_tensor("qt", [D, S], F32R, kind="ExternalInput")
    kt_d = nc.dram_tensor("kt", [D, S], F32R, kind="ExternalInput")
    vt_d = nc.dram_tensor("vt", [D, S], F32R, kind="ExternalInput")
    wqt_d = nc.dram_tensor("wqt", [D, HD], F32R, kind="ExternalInput")
    wkt_d = nc.dram_tensor("wkt", [D, HD], F32R, kind="ExternalInput")
    wvt_d = nc.dram_tensor("wvt", [D, HD], F32R, kind="ExternalInput")
    wot_d = nc.dram_tensor("wot", [HD, D], F32R, kind="ExternalInput")
    bqt_d = nc.dram_tensor("bqt", [HD, 1], F32, kind="ExternalInput")
    bkt_d = nc.dram_tensor("bkt", [HD, 1], F32, kind="ExternalInput")
    bvr_d = nc.dram_tensor("bvr", [1, HD], F32, kind="ExternalInput")
    bor_d = nc.dram_tensor("bor", [1, D], F32, kind="ExternalInput")
    idnr_d = nc.dram_tensor("idnr", [128, 128], F32R, kind="ExternalInput")
    if causal:
        bias_d = nc.dram_tensor("biasd", [4, 128, 512], F32R,
                                kind="ExternalInput")
    else:
        bias_d = nc.dram_tensor("biasf", [SB, 128, S], F32R,
                                kind="ExternalInput")
    attn_o = nc.dram_tensor("attn_out", [HC, S, S], F32R,
                            kind="ExternalOutput")
    out_o = nc.dram_tensor("out_part", [S, D], F32, kind="ExternalOutput")

    norm_eng = {"gpsimd": nc.gpsimd, "vector": nc.vector,
                "scalar": nc.scalar}[NORM_ENGINE]

    with tile.TileContext(nc) as tc:
        with tc.tile_pool(name="const", bufs=1) as const, \
             tc.tile_pool(name="persist", bufs=1) as persist, \
             tc.tile_pool(name="psS", bufs=4, space="PSUM") as psS, \
             tc.tile_pool(name="psT", bufs=2, space="PSUM") as psT, \
             tc.tile_pool(name="psM", bufs=2, space="PSUM") as psM:
            # ---- constants ----
            idnr_t = const.tile([128, 128], F32R)
            nc.sync.dma_start(out=idnr_t, in_=idnr_d.ap())
            bvb_t = const.tile([128, HD], F32)
            nc.sync.dma_start(out=bvb_t, in_=bvr_d.ap().to_broadcast([128, HD]))
            bob_t = const.tile([128, D], F32)
            nc.sync.dma_start(out=bob_t, in_=bor_d.ap().to_broadcast([128, D]))
            bq_t, bk_t = [], []
            for p in range(2):
                bq = const.tile([128, 1], F32, tag=f"bq{p}")
                nc.sync.dma_start(out=bq, in_=bqt_d.ap()[p * 128:(p + 1) * 128, :])
                bq_t.append(bq)
                bk = const.tile([128, 1], F32, tag=f"bk{p}")
                nc.sync.dma_start(out=bk, in_=bkt_d.ap()[p * 128:(p + 1) * 128, :])
                bk_t.append(bk)
            if causal:
                biasd_t = const.tile([128, 4, 512], F32R)
                nc.sync.dma_start(out=biasd_t,
                                  in_=bias_d.ap().rearrange("v p c -> p v c"))

            # ---- persistent per-core activations ----
            qT_t = [persist.tile([128, S], F32R, tag=f"qT{p}", name=f"qT{p}")
                    for p in range(2)]
            kT_t = [persist.tile([128, S], F32R, tag=f"kT{p}", name=f"kT{p}")
                    for p in range(2)]
            v_t = [persist.tile([128, HD], BF16, tag=f"v{s}", name=f"v{s}")
                   for s in range(SB)]
            cxT_t = [persist.tile([128, S], F32R, tag=f"cxT{p}", name=f"cxT{p}")
                     for p in range(2)]

            # ================= Phase 1: projections =================
            with tc.tile_pool(name="wts", bufs=1) as wts, \
                 tc.tile_pool(name="stream", bufs=1) as stream:
                w_t = {}
                for nm, dram in (("q", wqt_d), ("k", wkt_d), ("v", wvt_d)):
                    w = wts.tile([128, 8, HD], F32R, tag=f"w{nm}")
                    nc.sync.dma_start(
                        out=w, in_=dram.ap().rearrange("(e p) d -> p e d", p=128))
                    w_t[nm] = w

                for nm, src in (("q", qt_d), ("k", kt_d), ("v", vt_d)):
                    st = []
                    for e in range(8):
                        t = stream.tile([128, S], F32R, tag=f"st{e}",
                                        name=f"st_{nm}{e}")
                        nc.sync.dma_start(
                            out=t, in_=src.ap()[e * 128:(e + 1) * 128, :])
                        st.append(t)
                    if nm in ("q", "k"):
                        dst = qT_t if nm == "q" else kT_t
                        bias = bq_t if nm == "q" else bk_t
                        for p in range(2):
                            for c in range(4):
                                ps = psS.tile([128, 512], F32, tag="ps512")
                                for e in range(8):
                                    nc.tensor.matmul(
                                        ps,
                                        w_t[nm][:, e, p * 128:(p + 1) * 128],
                                        st[e][:, c * 512:(c + 1) * 512],
                                        start=(e == 0), stop=(e == 7))
                                nc.vector.tensor_scalar_add(
                                    dst[p][:, c * 512:(c + 1) * 512], ps, bias[p])
                    else:
                        for s in range(SB):
                            ps = psM.tile([128, HD], F32, tag="psm")
                            for e in range(8):
                                nc.tensor.matmul(
                                    ps, st[e][:, s * 128:(s + 1) * 128],
                                    w_t["v"][:, e, :],
                                    start=(e == 0), stop=(e == 7))
                            nc.vector.tensor_add(v_t[s], ps, bvb_t)

            # ================= Phase 2: attention =================
            with tc.tile_pool(name="atl", bufs=8) as atl, \
                 tc.tile_pool(name="atT", bufs=1) as atT, \
                 tc.tile_pool(name="sml", bufs=8) as sml, \
                 tc.tile_pool(name="bstr", bufs=2) as bstr:
                aT_t = [atT.tile([128, 512], BF16, tag=f"aT{j}", name=f"aT{j}")
                        for j in range(SB)]
                for bh in range(HC):
                    p, w = bh // 2, bh % 2
                    plo = w * 64
                    for c in range(4):
                        jmax = (4 * c + 3) if causal else (SB - 1)
                        at4 = []
                        for r in range(4):
                            i = 4 * c + r
                            nch = (i // 4 + 1) if causal else 4
                            at = atl.tile([128, S], F32R, tag="at")
                            sums = sml.tile([128, 4], F32, tag="sums")
                            if not causal:
                                bf = bstr.tile([128, S], F32R, tag="bf")
                                nc.sync.dma_start(out=bf, in_=bias_d.ap()[i])
                            for cc in range(nch):
                                diag = causal and (cc == nch - 1)
                                cw = ((i % 4 + 1) * 128) if diag else 512
                                ps = psS.tile([128, 512], F32, tag="ps512")
                                if diag or not causal:
                                    brhs = (biasd_t[:, i % 4, :cw] if causal
                                            else bf[:, cc * 512:cc * 512 + cw])
                                    nc.tensor.matmul(
                                        ps[:, :cw], idnr_t, brhs,
                                        start=True, stop=False)
                                nc.tensor.matmul(
                                    ps[:, :cw],
                                    qT_t[p][plo:plo + 64, i * 128:(i + 1) * 128],
                                    kT_t[p][plo:plo + 64,
                                            cc * 512:cc * 512 + cw],
                                    start=(causal and not diag), stop=True)
                                nc.scalar.activation(
                                    out=at[:, cc * 512:cc * 512 + cw],
                                    in_=ps[:, :cw],
                                    func=AF.Exp, scale=0.125,
                                    accum_out=sums[:, cc:cc + 1])
                            rcp = sml.tile([128, 1], F32, tag="rcp")
                            if nch > 1:
                                rs = sml.tile([128, 1], F32, tag="rs")
                                nc.vector.reduce_sum(out=rs, in_=sums[:, :nch],
                                                     axis=AX.X)
                                nc.vector.reciprocal(rcp, rs)
                            else:
                                nc.vector.reciprocal(rcp, sums[:, :1])
                            Lw = (i + 1) * 128 if causal else S
                            for cc in range(nch):
                                cw = min(512, Lw - cc * 512)
                                norm_eng.tensor_scalar_mul(
                                    at[:, cc * 512:cc * 512 + cw],
                                    at[:, cc * 512:cc * 512 + cw], rcp)
                            nc.sync.dma_start(
                                out=attn_o.ap()[bh, i * 128:(i + 1) * 128, :Lw],
                                in_=at[:, :Lw])
                            at4.append(at)
                        # transposes into aT tiles, then ctx matmuls
                        for j in range(jmax + 1):
                            pst = psT.tile([128, 512], F32R, tag="pst")
                            for r in range(4):
                                i = 4 * c + r
                                if causal and j > i:
                                    nc.vector.memset(
                                        pst[:, r * 128:(r + 1) * 128]
                                        .bitcast(F32), 0.0)
                                else:
                                    nc.tensor.transpose(
                                        pst[:, r * 128:(r + 1) * 128],
                                        at4[r][:, j * 128:(j + 1) * 128], idnr_t)
                            nc.vector.tensor_copy(aT_t[j], pst)
                        pc = psM.tile([64, 512], F32, tag="psm")
                        for j in range(jmax + 1):
                            nc.tensor.matmul(
                                pc, v_t[j][:, bh * 64:(bh + 1) * 64], aT_t[j],
                                start=(j == 0), stop=(j == jmax))
                        nc.vector.tensor_copy(
                            cxT_t[p][plo:plo + 64, c * 512:(c + 1) * 512], pc)

            # ================= Phase 3: output projection =================
            with tc.tile_pool(name="osb", bufs=3) as osb, \
                 tc.tile_pool(name="wos", bufs=1) as wos:
                wo_t = []
                for p in range(2):
                    wt = wos.tile([128, D], F32R, tag=f"wo{p}")
                    nc.sync.dma_start(out=wt,
                                      in_=wot_d.ap()[p * 128:(p + 1) * 128, :])
                    wo_t.append(wt)
                for s in range(SB):
                    ot = osb.tile([128, D], F32, tag="ot")
                    for oc in range(2):
                        ps = psS.tile([128, 512], F32, tag="ps512")
                        for p in range(2):
                            nc.tensor.matmul(
                                ps, cxT_t[p][:, s * 128:(s + 1) * 128],
                                wo_t[p][:, oc * 512:(oc + 1) * 512],
                                start=(p == 0), stop=(p == 1))
                        nc.vector.tensor_add(
                            ot[:, oc * 512:(oc + 1) * 512], ps,
                            bob_t[:, oc * 512:(oc + 1) * 512])
                    nc.sync.dma_start(out=out_o.ap()[s * 128:(s + 1) * 128, :],
                                      in_=ot)

    nc.compile()
    return nc


def _causal_bias_tiles():
    v = np.zeros((4, 128, 512), np.float32)
    blk = np.triu(np.full((128, 128), -8e9, np.float32), 1)
    for r in range(4):
        v[r, :, r * 128:(r + 1) * 128] = blk
        v[r, :, (r + 1) * 128:] = -8e9
    return v


def kernel(Q, K, V, mask, Wq, bq, Wk, bk, Wv, bv, Wo, bo):
    global LAST_RES
    Q, K, V = (np.asarray(x, np.float32) for x in (Q, K, V))
    mask = np.asarray(mask)
    Wq, bq, Wk, bk, Wv, bv, Wo, bo = (
        np.asarray(x, np.float32) for x in (Wq, bq, Wk, bk, Wv, bv, Wo, bo))

    m2 = np.asarray(mask).reshape(S, S)
    causal = bool(np.array_equal(m2 != 0, np.tril(np.ones((S, S), bool))))

    if causal not in _PROG_CACHE:
        _PROG_CACHE[causal] = _build_program(causal)
    nc = _PROG_CACHE[causal]

    idn_np = np.eye(128, dtype=np.float32)
    if causal:
        bias_np = _causal_bias_tiles()
    else:
        bias_np = np.where(m2[None] == 0, np.float32(-8e9), np.float32(0.0))
        bias_np = np.ascontiguousarray(
            bias_np.reshape(SB, 128, S)).astype(np.float32)

    in_maps = []
    for c in range(N_CORES):
        b = c // (N_CORES // B)
        g = c % (N_CORES // B)
        hs, he = g * HD, (g + 1) * HD
        in_maps.append({
            "qt": np.ascontiguousarray(Q[b].T),
            "kt": np.ascontiguousarray(K[b].T),
            "vt": np.ascontiguousarray(V[b].T),
            "wqt": np.ascontiguousarray(Wq[hs:he].T),
            "wkt": np.ascontiguousarray(Wk[hs:he].T),
            "wvt": np.ascontiguousarray(Wv[hs:he].T),
            "wot": np.ascontiguousarray(Wo[:, hs:he].T),
            "bqt": np.ascontiguousarray(bq[hs:he, None]),
            "bkt": np.ascontiguousarray(bk[hs:he, None]),
            "bvr": np.ascontiguousarray(bv[None, hs:he]),
            "bor": (bo[None, :] if g == 0 else
                    np.zeros((1, D), np.float32)),
            "idnr": idn_np,
            ("biasd" if causal else "biasf"): bias_np,
        })

    res = run_bass_kernel_spmd(nc, in_maps, core_ids=list(range(N_CORES)),
                               trace=TRACE)
    LAST_RES = res

    attn = np.empty((B, H, S, S), np.float32)
    out = np.zeros((B, S, D), np.float32)
    for c in range(N_CORES):
        b = c // (N_CORES // B)
        g = c % (N_CORES // B)
        attn[b, g * HC:(g + 1) * HC] = res.results[c]["attn_out"]
        out[b] += res.results[c]["out_part"]
    return out, attn


# revision 10
# speedup vs baseline: 3.0162x; 3.0162x over previous
"""Multi-head attention (B=2, S=2048, D=1024, H=16) on 8 Trainium2 NeuronCores.

Sharding: core = (batch, head-group-of-4). Cores 0-3 -> b=0, head groups 0-3;
cores 4-7 -> b=1. Column-parallel Wq/Wk/Wv (each core projects its 256 head
dims), row-parallel Wo (each core produces a partial [S, D] output summed on
host). The attention-probability output attn [B,H,S,S] is written directly by
each core for its 4 (b,h) pairs.

Fast path: if the mask equals the causal tril mask, only lower-triangle blocks
are computed; upper-triangle stays zero via the runtime's pre-zeroed output
buffers. Masking is applied by accumulating a -8e9 bias into the scores PSUM
through an identity matmul, so exp() produces exact zeros and row sums (taken
via the activation accum_out) are correct. A generic path streams a full
[S, S] additive bias built from the mask on the host.

Matmuls run in float32r (~11-bit mantissa, 1 cyc/row at N>=256); the
attn-probability tiles are float32r end-to-end (PE transpose at 1.5 cyc/row);
the attn^T @ v context path runs in bf16. Row normalization runs on the
otherwise-idle GPSIMD engine; PSUM->SBUF traffic runs on DVE.
"""
import sys

for _p in ("/opt/trn_rl_repo", "/root/.axon_site/_ro/trn_rl_repo"):
    if _p not in sys.path:
        sys.path.append(_p)

import numpy as np
import ml_dtypes
import concourse.bacc as bacc
import concourse.tile as tile
from concourse import mybir
from concourse.bass_utils import run_bass_kernel_spmd

B, S, D, H = 2, 2048, 1024, 16
DK = D // H  # 64
N_CORES = 8
HC = H // (N_CORES // B)  # 4 heads per core
HD = HC * DK  # 256 head dims per core
SB = S // 128  # 16 q/k blocks
F32 = mybir.dt.float32
F32R = mybir.dt.float32r
BF16 = mybir.dt.bfloat16
AF = mybir.ActivationFunctionType
AX = mybir.AxisListType

TRACE = False
NORM_ENGINE = "vector"  # "gpsimd" | "vector" | "scalar"
LAST_RES = None
_PROG_CACHE = {}


def _build_program(causal: bool):
    nc = bacc.Bacc("TRN2", target_bir_lowering=False, debug=False,
                   num_devices=N_CORES)

    qt_d = nc.dram_tensor("qt", [D, S], F32R, kind="ExternalInput")
    kt_d = nc.dram_tensor("kt", [D, S], F32R, kind="ExternalInput")
    vt_d = nc.dram_tensor("vt", [D, S], F32R, kind="ExternalInput")
    wqt_d = nc.dram_tensor("wqt", [D, HD], F32R, kind="ExternalInput")
    wkt_d = nc.dram_tensor("wkt", [D, HD], F32R, kind="ExternalInput")
    wvt_d = nc.dram_tensor("wvt", [D, HD], F32R, kind="ExternalInput")
    wot_d = nc.dram_tensor("wot", [HD, D], F32R, kind="ExternalInput")
    bqt_d = nc.dram_tensor("bqt", [HD, 1], F32, kind="ExternalInput")
    bkt_d = nc.dram_tensor("bkt", [HD, 1], F32, kind="ExternalInput")
    bvr_d = nc.dram_tensor("bvr", [1, HD], F32, kind="ExternalInput")
    bor_d = nc.dram_tensor("bor", [1, D], F32, kind="ExternalInput")
    idnr_d = nc.dram_tensor("idnr", [128, 128], F32R, kind="ExternalInput")
    idnb_d = nc.dram_tensor("idnb", [128, 128], BF16, kind="ExternalInput")
    if causal:
        bias_d = nc.dram_tensor("biasd", [4, 128, 512], F32R,
                                kind="ExternalInput")
    else:
        bias_d = nc.dram_tensor("biasf", [SB, 128, S], F32R,
                                kind="ExternalInput")
    attn_o = nc.dram_tensor("attn_out", [HC, S, S], BF16,
                            kind="ExternalOutput")
    out_o = nc.dram_tensor("out_part", [S, D], F32, kind="ExternalOutput")

    norm_eng = {"gpsimd": nc.gpsimd, "vector": nc.vector,
                "scalar": nc.scalar}[NORM_ENGINE]

    with tile.TileContext(nc) as tc:
        with tc.tile_pool(name="const", bufs=1) as const, \
             tc.tile_pool(name="persist", bufs=1) as persist, \
             tc.tile_pool(name="psS", bufs=4, space="PSUM") as psS, \
             tc.tile_pool(name="psT", bufs=2, space="PSUM") as psT, \
             tc.tile_pool(name="psM", bufs=2, space="PSUM") as psM:
            # ---- constants ----
            idnr_t = const.tile([128, 128], F32R)
            nc.sync.dma_start(out=idnr_t, in_=idnr_d.ap())
            idnb_t = const.tile([128, 128], BF16)
            nc.sync.dma_start(out=idnb_t, in_=idnb_d.ap())
            bvb_t = const.tile([128, HD], F32)
            nc.sync.dma_start(out=bvb_t, in_=bvr_d.ap().to_broadcast([128, HD]))
            bob_t = const.tile([128, D], F32)
            nc.sync.dma_start(out=bob_t, in_=bor_d.ap().to_broadcast([128, D]))
            bq_t, bk_t = [], []
            for p in range(2):
                bq = const.tile([128, 1], F32, tag=f"bq{p}")
                nc.sync.dma_start(out=bq, in_=bqt_d.ap()[p * 128:(p + 1) * 128, :])
                bq_t.append(bq)
                bk = const.tile([128, 1], F32, tag=f"bk{p}")
                nc.sync.dma_start(out=bk, in_=bkt_d.ap()[p * 128:(p + 1) * 128, :])
                bk_t.append(bk)
            if causal:
                biasd_t = const.tile([128, 4, 512], F32R)
                nc.sync.dma_start(out=biasd_t,
                                  in_=bias_d.ap().rearrange("v p c -> p v c"))

            # ---- persistent per-core activations ----
            qT_t = [persist.tile([128, S], F32R, tag=f"qT{p}", name=f"qT{p}")
                    for p in range(2)]
            kT_t = [persist.tile([128, S], F32R, tag=f"kT{p}", name=f"kT{p}")
                    for p in range(2)]
            v_t = [persist.tile([128, HD], BF16, tag=f"v{s}", name=f"v{s}")
                   for s in range(SB)]
            cxT_t = [persist.tile([128, S], F32R, tag=f"cxT{p}", name=f"cxT{p}")
                     for p in range(2)]

            # ================= Phase 1: projections =================
            with tc.tile_pool(name="wts", bufs=1) as wts, \
                 tc.tile_pool(name="stream", bufs=1) as stream:
                w_t = {}
                for nm, dram in (("q", wqt_d), ("k", wkt_d), ("v", wvt_d)):
                    w = wts.tile([128, 8, HD], F32R, tag=f"w{nm}")
                    nc.sync.dma_start(
                        out=w, in_=dram.ap().rearrange("(e p) d -> p e d", p=128))
                    w_t[nm] = w

                for nm, src in (("q", qt_d), ("k", kt_d), ("v", vt_d)):
                    st = []
                    for e in range(8):
                        t = stream.tile([128, S], F32R, tag=f"st{e}",
                                        name=f"st_{nm}{e}")
                        nc.sync.dma_start(
                            out=t, in_=src.ap()[e * 128:(e + 1) * 128, :])
                        st.append(t)
                    if nm in ("q", "k"):
                        dst = qT_t if nm == "q" else kT_t
                        bias = bq_t if nm == "q" else bk_t
                        for p in range(2):
                            for c in range(4):
                                ps = psS.tile([128, 512], F32, tag="ps512")
                                for e in range(8):
                                    nc.tensor.matmul(
                                        ps,
                                        w_t[nm][:, e, p * 128:(p + 1) * 128],
                                        st[e][:, c * 512:(c + 1) * 512],
                                        start=(e == 0), stop=(e == 7))
                                nc.vector.tensor_scalar_add(
                                    dst[p][:, c * 512:(c + 1) * 512], ps, bias[p])
                    else:
                        for s in range(SB):
                            ps = psM.tile([128, HD], F32, tag="psm")
                            for e in range(8):
                                nc.tensor.matmul(
                                    ps, st[e][:, s * 128:(s + 1) * 128],
                                    w_t["v"][:, e, :],
                                    start=(e == 0), stop=(e == 7))
                            nc.vector.tensor_add(v_t[s], ps, bvb_t)

            # ================= Phase 2: attention =================
            with tc.tile_pool(name="atl", bufs=8) as atl, \
                 tc.tile_pool(name="atT", bufs=1) as atT, \
                 tc.tile_pool(name="sml", bufs=8) as sml, \
                 tc.tile_pool(name="bstr", bufs=2) as bstr:
                aT_t = [atT.tile([128, 512], BF16, tag=f"aT{j}", name=f"aT{j}")
                        for j in range(SB)]
                for bh in range(HC):
                    p, w = bh // 2, bh % 2
                    plo = w * 64
                    for c in range(4):
                        jmax = (4 * c + 3) if causal else (SB - 1)
                        at4 = []
                        for r in range(4):
                            i = 4 * c + r
                            nch = (i // 4 + 1) if causal else 4
                            at = atl.tile([128, S], BF16, tag="at")
                            sums = sml.tile([128, 4], F32, tag="sums")
                            if not causal:
                                bf = bstr.tile([128, S], F32R, tag="bf")
                                nc.sync.dma_start(out=bf, in_=bias_d.ap()[i])
                            for cc in range(nch):
                                diag = causal and (cc == nch - 1)
                                cw = ((i % 4 + 1) * 128) if diag else 512
                                ps = psS.tile([128, 512], F32, tag="ps512")
                                if diag or not causal:
                                    brhs = (biasd_t[:, i % 4, :cw] if causal
                                            else bf[:, cc * 512:cc * 512 + cw])
                                    nc.tensor.matmul(
                                        ps[:, :cw], idnr_t, brhs,
                                        start=True, stop=False)
                                nc.tensor.matmul(
                                    ps[:, :cw],
                                    qT_t[p][plo:plo + 64, i * 128:(i + 1) * 128],
                                    kT_t[p][plo:plo + 64,
                                            cc * 512:cc * 512 + cw],
                                    start=(causal and not diag), stop=True)
                                nc.scalar.activation(
                                    out=at[:, cc * 512:cc * 512 + cw],
                                    in_=ps[:, :cw],
                                    func=AF.Exp, scale=0.125,
                                    accum_out=sums[:, cc:cc + 1])
                            rcp = sml.tile([128, 1], F32, tag="rcp")
                            if nch > 1:
                                rs = sml.tile([128, 1], F32, tag="rs")
                                nc.vector.reduce_sum(out=rs, in_=sums[:, :nch],
                                                     axis=AX.X)
                                nc.vector.reciprocal(rcp, rs)
                            else:
                                nc.vector.reciprocal(rcp, sums[:, :1])
                            Lw = (i + 1) * 128 if causal else S
                            for cc in range(nch):
                                cw = min(512, Lw - cc * 512)
                                norm_eng.tensor_scalar_mul(
                                    at[:, cc * 512:cc * 512 + cw],
                                    at[:, cc * 512:cc * 512 + cw], rcp)
                            nc.sync.dma_start(
                                out=attn_o.ap()[bh, i * 128:(i + 1) * 128, :Lw],
                                in_=at[:, :Lw])
                            at4.append(at)
                        # transposes into aT tiles, then ctx matmuls
                        for j in range(jmax + 1):
                            pst = psT.tile([128, 512], BF16, tag="pst")
                            r0 = max(0, j - 4 * c) if causal else 0
                            for r in range(r0, 4):
                                nc.tensor.transpose(
                                    pst[:, r * 128:(r + 1) * 128],
                                    at4[r][:, j * 128:(j + 1) * 128], idnb_t)
                            if r0:
                                nc.vector.memset(aT_t[j][:, :r0 * 128], 0.0)
                            nc.vector.tensor_copy(aT_t[j][:, r0 * 128:],
                                                  pst[:, r0 * 128:])
                        pc = psM.tile([64, 512], F32, tag="psm")
                        for j in range(jmax + 1):
                            nc.tensor.matmul(
                                pc, v_t[j][:, bh * 64:(bh + 1) * 64], aT_t[j],
                                start=(j == 0), stop=(j == jmax))
                        nc.vector.tensor_copy(
                            cxT_t[p][plo:plo + 64, c * 512:(c + 1) * 512], pc)

            # ================= Phase 3: output projection =================
            with tc.tile_pool(name="osb", bufs=3) as osb, \
                 tc.tile_pool(name="wos", bufs=1) as wos:
                wo_t = []
                for p in range(2):
                    wt = wos.tile([128, D], F32R, tag=f"wo{p}")
                    nc.sync.dma_start(out=wt,
                                      in_=wot_d.ap()[p * 128:(p + 1) * 128, :])
                    wo_t.append(wt)
                for s in range(SB):
                    ot = osb.tile([128, D], F32, tag="ot")
                    for oc in range(2):
                        ps = psS.tile([128, 512], F32, tag="ps512")
                        for p in range(2):
                            nc.tensor.matmul(
                                ps, cxT_t[p][:, s * 128:(s + 1) * 128],
                                wo_t[p][:, oc * 512:(oc + 1) * 512],
                                start=(p == 0), stop=(p == 1))
                        nc.vector.tensor_add(
                            ot[:, oc * 512:(oc + 1) * 512], ps,
                            bob_t[:, oc * 512:(oc + 1) * 512])
                    nc.sync.dma_start(out=out_o.ap()[s * 128:(s + 1) * 128, :],
                                      in_=ot)

    nc.compile()
    return nc


def _causal_bias_tiles():
    v = np.zeros((4, 128, 512), np.float32)
    blk = np.triu(np.full((128, 128), -8e9, np.float32), 1)
    for r in range(4):
        v[r, :, r * 128:(r + 1) * 128] = blk
        v[r, :, (r + 1) * 128:] = -8e9
    return v


def kernel(Q, K, V, mask, Wq, bq, Wk, bk, Wv, bv, Wo, bo):
    global LAST_RES
    Q, K, V = (np.asarray(x, np.float32) for x in (Q, K, V))
    mask = np.asarray(mask)
    Wq, bq, Wk, bk, Wv, bv, Wo, bo = (
        np.asarray(x, np.float32) for x in (Wq, bq, Wk, bk, Wv, bv, Wo, bo))

    m2 = np.asarray(mask).reshape(S, S)
    causal = bool(np.array_equal(m2 != 0, np.tril(np.ones((S, S), bool))))

    if causal not in _PROG_CACHE:
        _PROG_CACHE[causal] = _build_program(causal)
    nc = _PROG_CACHE[causal]

    idn_np = np.eye(128, dtype=np.float32)
    idnb_np = np.eye(128, dtype=np.float32).astype(ml_dtypes.bfloat16)
    if causal:
        bias_np = _causal_bias_tiles()
    else:
        bias_np = np.where(m2[None] == 0, np.float32(-8e9), np.float32(0.0))
        bias_np = np.ascontiguousarray(
            bias_np.reshape(SB, 128, S)).astype(np.float32)

    in_maps = []
    for c in range(N_CORES):
        b = c // (N_CORES // B)
        g = c % (N_CORES // B)
        hs, he = g * HD, (g + 1) * HD
        in_maps.append({
            "qt": np.ascontiguousarray(Q[b].T),
            "kt": np.ascontiguousarray(K[b].T),
            "vt": np.ascontiguousarray(V[b].T),
            "wqt": np.ascontiguousarray(Wq[hs:he].T),
            "wkt": np.ascontiguousarray(Wk[hs:he].T),
            "wvt": np.ascontiguousarray(Wv[hs:he].T),
            "wot": np.ascontiguousarray(Wo[:, hs:he].T),
            "bqt": np.ascontiguousarray(bq[hs:he, None]),
            "bkt": np.ascontiguousarray(bk[hs:he, None]),
            "bvr": np.ascontiguousarray(bv[None, hs:he]),
            "bor": (bo[None, :] if g == 0 else
                    np.zeros((1, D), np.float32)),
            "idnr": idn_np,
            "idnb": idnb_np,
            ("biasd" if causal else "biasf"): bias_np,
        })

    res = run_bass_kernel_spmd(nc, in_maps, core_ids=list(range(N_CORES)),
                               trace=TRACE)
    LAST_RES = res

    attn = np.empty((B, H, S, S), np.float32)
    out = np.zeros((B, S, D), np.float32)
    for c in range(N_CORES):
        b = c // (N_CORES // B)
        g = c % (N_CORES // B)
        attn[b, g * HC:(g + 1) * HC] = res.results[c]["attn_out"]
        out[b] += res.results[c]["out_part"]
    return out, attn
